# revision 29
# baseline (speedup 1.0000x reference)
"""Trainium2 Bass kernel: 3-layer GAT message passing, 8-core SPMD.

v4: feature-AllGather + local full-table build.
  - AllGather the per-layer feature pieces (128/256/128 cols) in 4 chunks,
    not the fat [h|as|ad] tables (2.2x fewer collective bytes).
  - Every core builds the full-N gather table locally (PE matmuls), so edge
    gathers depend only on local DMA writes, never on collectives.
  - Per-edge dma_gather on 4 SWDGE queues round-robin; host-precomputed fp8
    one-hot scatter/expand matrices; own-shard mini-table for self rows + ad.
"""

import math
from dataclasses import dataclass, field

import numpy as np
import ml_dtypes

import concourse.bass as bass
import concourse.bacc as bacc
import concourse.mybir as mybir
import concourse.tile as tile
from concourse import library_config

F32 = mybir.dt.float32
BF16 = mybir.dt.bfloat16
I16 = mybir.dt.int16
F8 = mybir.dt.float8e4
AF = mybir.ActivationFunctionType
ALU = mybir.AluOpType
NEG_SLOPE = 0.2

bf = ml_dtypes.bfloat16
f8 = np.dtype(ml_dtypes.float8_e4m3)


@dataclass
class Cfg:
    N: int = 50000
    n_cores: int = 8
    IN: int = 256
    HID: int = 128
    OUT: int = 128
    K_FUSE: int = 8
    n_queues: int = 4
    n_chunks: int = 4
    NSPLIT: int = 25000   # lo/hi gather table split (int16 idx limit)

    @property
    def n_loc(self):
        return self.N // self.n_cores

    @property
    def n_win(self):
        return (self.n_loc + 127) // 128

    @property
    def n_win_g(self):  # global windows per rank (same as n_win)
        return self.n_win

    def win_size(self, t):
        return min(128, self.n_loc - t * 128)

    @property
    def chunk_bounds(self):
        W, K = self.n_win, self.n_chunks
        base, rem = divmod(W, K)
        b = [0]
        for i in range(K):
            b.append(b[-1] + base + (1 if i < rem else 0))
        return b

    def chunk_of_win(self, t):
        b = self.chunk_bounds
        for k in range(self.n_chunks):
            if t < b[k + 1]:
                return k
        raise ValueError(t)

    def chunk_rows(self, k):
        b = self.chunk_bounds
        return min(b[k + 1] * 128, self.n_loc) - b[k] * 128

    @property
    def layers(self):
        hid, out = self.HID, self.OUT
        ls = []
        for (H, C, in_ch) in ((4, hid // 2, hid), (2, out, 2 * hid), (1, out, out)):
            HC = H * C
            row = HC + 2 * H
            row_pad = ((row * 2 + 255) // 256) * 256 // 2
            ls.append(dict(H=H, C=C, in_ch=in_ch, HC=HC, row=row_pad,
                           as_off=HC, ad_off=HC + H))
        return ls


# ---------------------------------------------------------------- host plan

@dataclass
class Plan:
    T_lo: list
    T_hi: list
    idx_lo: list
    idx_hi: list
    cols_lo: int = 0
    cols_hi: int = 0
    win_tile_off: list = field(default_factory=list)
    win_lo_off: list = field(default_factory=list)
    win_hi_off: list = field(default_factory=list)
    oT: list = field(default_factory=list)
    oD: list = field(default_factory=list)
    NT: int = 0
    MAXT: int = 0


def plan_edges(edge_index, cfg: Cfg) -> Plan:
    N, C = cfg.N, cfg.n_cores
    n_loc = cfg.n_loc
    W = cfg.n_win
    NS = cfg.NSPLIT

    src = np.asarray(edge_index[0], np.int64)   # self loops handled by a
    dst = np.asarray(edge_index[1], np.int64)   # direct per-window load
    core = dst // n_loc
    win = (dst % n_loc) // 128
    dloc = (dst % n_loc) % 128
    lo = src < NS

    buckets = {}
    for c in range(C):
        m_c = core == c
        for t in range(W):
            m_t = m_c & (win == t)
            for half in (0, 1):
                m = m_t & (lo if half == 0 else ~lo)
                si = src[m] - (0 if half == 0 else NS)
                dl = dloc[m]
                o = np.argsort(si, kind="stable")
                buckets[(c, t, half)] = (si[o], dl[o])

    T_lo = [max(math.ceil(len(buckets[(c, t, 0)][0]) / 128) for c in range(C))
            for t in range(W)]
    T_hi = [max(math.ceil(len(buckets[(c, t, 1)][0]) / 128) for c in range(C))
            for t in range(W)]

    NT = sum(T_lo) + sum(T_hi) + W  # +1 self tile per window
    MAXT = max(T_lo[t] + T_hi[t] for t in range(W)) + 1
    p = Plan(T_lo=T_lo, T_hi=T_hi, idx_lo=[], idx_hi=[],
             cols_lo=max(sum(T_lo) * 8, 8), cols_hi=max(sum(T_hi) * 8, 8),
             NT=NT, MAXT=MAXT)
    off = olo = ohi = 0
    for t in range(W):
        p.win_tile_off.append(off)
        p.win_lo_off.append(olo)
        p.win_hi_off.append(ohi)
        off += T_lo[t] + T_hi[t] + 1
        olo += T_lo[t] * 8
        ohi += T_hi[t] * 8

    ar = np.arange(128, dtype=np.float32)
    for c in range(C):
        ilo = np.zeros((128, p.cols_lo), np.int16)
        ihi = np.zeros((128, p.cols_hi), np.int16)
        dcol = np.full((128, NT), -1.0, np.float32)
        for t in range(W):
            for half, (idx_arr, col_off, Tn) in enumerate(
                ((ilo, p.win_lo_off[t], T_lo[t]),
                 (ihi, p.win_hi_off[t], T_hi[t]))):
                if Tn == 0:
                    continue
                s, dl = buckets[(c, t, half)]
                n = Tn * 128
                si = np.zeros(n, np.int64)
                si[:len(s)] = s
                dli = np.full(n, -1.0, np.float32)
                dli[:len(dl)] = dl
                blk = si.astype(np.int16).reshape(Tn * 8, 16).T
                idx_arr[:, col_off:col_off + Tn * 8] = np.tile(blk, (8, 1))
                tb = p.win_tile_off[t] + (0 if half == 0 else T_lo[t])
                dcol[:, tb:tb + Tn] = dli.reshape(Tn, 128).T
            ts = p.win_tile_off[t] + T_lo[t] + T_hi[t]
            nn_w = cfg.win_size(t)
            selfcol = np.full(128, -1.0, np.float32)
            selfcol[:nn_w] = np.arange(nn_w)
            dcol[:, ts] = selfcol
        p.idx_lo.append(ilo)
        p.idx_hi.append(ihi)
        eq = (dcol[:, :, None] == ar[None, None, :])
        p.oT.append(np.ascontiguousarray(
            eq.reshape(128, NT * 128).astype(np.float32)).astype(f8))
        p.oD.append(np.ascontiguousarray(
            eq.transpose(2, 1, 0).reshape(128, NT * 128)
            .astype(np.float32)).astype(f8))
    return p


# ------------------------------------------------------------- host weights

def prep_weights(inp, cfg: Cfg):
    out = {}
    for li, (wk, ak, dk) in enumerate(
            (("g1_W", "g1_as", "g1_ad"), ("g2_W", "g2_as", "g2_ad"),
             ("g3_W", "g3_as", "g3_ad"))):
        L = cfg.layers[li]
        Wm = np.asarray(inp[wk], np.float32)
        a_s = np.asarray(inp[ak], np.float32)
        a_d = np.asarray(inp[dk], np.float32)
        H, Cc = L["H"], L["C"]
        U_s = np.stack([Wm[:, h * Cc:(h + 1) * Cc] @ a_s[h] for h in range(H)], 1)
        U_d = np.stack([Wm[:, h * Cc:(h + 1) * Cc] @ a_d[h] for h in range(H)], 1)
        out[f"WG{li+1}"] = np.concatenate([Wm, U_s, U_d], 1).astype(bf)
        out[f"bG{li+1}"] = np.ascontiguousarray(np.broadcast_to(
            np.asarray(inp[f"g{li+1}_b"], np.float32)[None, :],
            (128, len(inp[f"g{li+1}_b"]))))
    out["Wm1"] = np.asarray(inp["W1"], np.float32).astype(bf)
    out["Wm2"] = np.asarray(inp["W2"], np.float32).astype(bf)
    out["b1c"] = np.ascontiguousarray(np.asarray(inp["b1"], np.float32)[:, None])
    out["b2c"] = np.ascontiguousarray(np.asarray(inp["b2"], np.float32)[:, None])
    out["ident"] = np.eye(128, dtype=np.float32).astype(bf)
    return out


# ---------------------------------------------------------------- builder

def build(nc, cfg: Cfg, p: Plan):
    W = cfg.n_win
    K = cfg.n_chunks
    R = cfg.n_cores
    n_loc = cfg.n_loc
    Ls = cfg.layers
    bounds = cfg.chunk_bounds
    rows = [cfg.chunk_rows(k) for k in range(K)]
    starts = [bounds[k] * 128 for k in range(K)]
    MAXT = p.MAXT
    MAXHC = max(L["HC"] for L in Ls)
    MAXH = max(L["H"] for L in Ls)
    pcols = [cfg.HID, Ls[0]["HC"], Ls[1]["C"]]  # feature cols per layer input

    def din(name, shape, dt):
        return nc.dram_tensor(name, list(shape), dt, kind="ExternalInput")

    xT = din("xT", (cfg.IN, n_loc), BF16)
    Wm1 = din("Wm1", (cfg.IN, cfg.HID), BF16)
    Wm2 = din("Wm2", (cfg.HID, cfg.HID), BF16)
    b1c = din("b1c", (cfg.HID, 1), F32)
    b2c = din("b2c", (cfg.HID, 1), F32)
    WG = [din(f"WG{i+1}", (Ls[i]["in_ch"], Ls[i]["HC"] + 2 * Ls[i]["H"]), BF16)
          for i in range(3)]
    bG = [din(f"bG{i+1}", (128, Ls[i]["HC"] if i == 0 else Ls[i]["C"]), F32)
          for i in range(3)]
    ident_d = din("ident", (128, 128), BF16)
    idx_lo_d = din("idx_lo", (128, p.cols_lo), I16)
    idx_hi_d = din("idx_hi", (128, p.cols_hi), I16)
    oT_d = din("oT", (128, p.NT * 128), F8)
    oD_d = din("oD", (128, p.NT * 128), F8)
    out_d = nc.dram_tensor("out", [n_loc, cfg.OUT], F32, kind="ExternalOutput")

    rep = [list(range(R))]

    with tile.TileContext(nc) as tc:
        with (
            tc.tile_pool(name="const", bufs=1) as cpool,
            tc.tile_pool(name="gat", bufs=3) as gpool,
            tc.tile_pool(name="oh", bufs=3) as opool,
            tc.tile_pool(name="msg", bufs=2) as mpool,
            tc.tile_pool(name="small", bufs=4) as spool,
            tc.tile_pool(name="nodes", bufs=3) as npool,
            tc.tile_pool(name="bld", bufs=4) as bpool,
            tc.tile_pool(name="psA", bufs=2, space="PSUM") as psA,
            tc.tile_pool(name="psB", bufs=2, space="PSUM") as psB,
            tc.tile_pool(name="psC", bufs=2, space="PSUM") as psC,
            tc.tile_pool(name="psD", bufs=2, space="PSUM") as psD,
            tc.tile_pool(name="dram", bufs=1, space="DRAM") as dpool,
        ):
            nc.gpsimd.load_library(library_config.mlp)
            gq = [0]  # round-robin SWDGE queue counter for gathers

            def load_const(handle, shape, dtp, tag):
                t = cpool.tile(list(shape), dtp, tag=tag, name=tag)
                nc.sync.dma_start(t[:], handle[:])
                return t

            ident = load_const(ident_d, (128, 128), BF16, "ident")
            idx_lo = load_const(idx_lo_d, (128, p.cols_lo), I16, "idx_lo")
            idx_hi = load_const(idx_hi_d, (128, p.cols_hi), I16, "idx_hi")
            wg_sb = []
            for i in range(3):
                L = Ls[i]
                kch = L["in_ch"] // 128
                t = cpool.tile([128, kch, L["HC"] + 2 * L["H"]], BF16,
                               tag=f"wg{i}", name=f"wg{i}")
                for k in range(kch):
                    nc.sync.dma_start(t[:, k, :], WG[i][k * 128:(k + 1) * 128, :])
                wg_sb.append(t)
            bg_sb = [load_const(bG[i], (128, Ls[i]["HC"] if i == 0 else Ls[i]["C"]),
                                F32, f"bg{i}") for i in range(3)]
            wm1 = cpool.tile([128, 2, cfg.HID], BF16, tag="wm1")
            for k in range(2):
                nc.sync.dma_start(wm1[:, k, :], Wm1[k * 128:(k + 1) * 128, :])
            wm2 = load_const(Wm2, (cfg.HID, cfg.HID), BF16, "wm2")
            b1s = load_const(b1c, (cfg.HID, 1), F32, "b1s")
            b2s = load_const(b2c, (cfg.HID, 1), F32, "b2s")

            # own-shard feature pieces (per chunk) + gathered full features
            piece_in = [[dpool.tile([pcols[i], rows[k]], BF16,
                                    tag=f"pin{i}_{k}", name=f"pin{i}_{k}")
                         for k in range(K)] for i in range(3)]
            piece_ag = [[dpool.tile([R, pcols[i], rows[k]], BF16,
                                    tag=f"pag{i}_{k}", name=f"pag{i}_{k}")
                         for k in range(K)] for i in range(3)]
            # full local gather tables + own-shard mini tables (self/ad rows)
            tblF = [dpool.tile([cfg.N, Ls[i]["row"]], BF16, tag=f"tblF{i}",
                               name=f"tblF{i}") for i in range(3)]
            tblS = [dpool.tile([n_loc, Ls[i]["row"]], BF16, tag=f"tblS{i}",
                               name=f"tblS{i}") for i in range(3)]

            def emit_ag(li, k):
                nc.gpsimd.collective_compute(
                    "AllGather", ALU.bypass, replica_groups=rep,
                    ins=[piece_in[li][k][:, :]], outs=[piece_ag[li][k][:, :]])

            def node_self(li, j):
                # own-shard mini table row build (self rows + ad source)
                L = Ls[li]
                kch = L["in_ch"] // 128
                NCOL = L["HC"] + 2 * L["H"]
                nn = cfg.win_size(j)
                kc = cfg.chunk_of_win(j)
                lcol = j * 128 - starts[kc]
                lh = npool.tile([128, kch, 128], BF16, tag="lh", name="lh")
                for k in range(kch):
                    nc.sync.dma_start(
                        lh[:, k, :nn],
                        piece_in[li][kc][k * 128:(k + 1) * 128,
                                         lcol:lcol + nn])
                ps = psC.tile([128, NCOL], F32, tag="mm", name="psn")
                for k in range(kch):
                    nc.tensor.matmul(ps[:nn, :], lh[:, k, :nn],
                                     wg_sb[li][:, k, :],
                                     start=(k == 0), stop=(k == kch - 1))
                tb = npool.tile([128, L["row"]], BF16, tag="tb", name="tb")
                nc.scalar.activation(tb[:nn, :NCOL], ps[:nn, :], AF.Copy)
                nc.sync.dma_start(
                    tblS[li][j * 128:j * 128 + nn, :NCOL], tb[:nn, :NCOL])

            def build_full(li, r, tt):
                # full-table row build for rank r, window tt (from piece_ag)
                L = Ls[li]
                kch = L["in_ch"] // 128
                NCOL = L["HC"] + 2 * L["H"]
                nn = cfg.win_size(tt)
                kc = cfg.chunk_of_win(tt)
                lcol = tt * 128 - starts[kc]
                lh = bpool.tile([128, kch, 128], BF16, tag="blh", name="blh")
                for k in range(kch):
                    nc.sync.dma_start(
                        lh[:, k, :nn],
                        piece_ag[li][kc][r, k * 128:(k + 1) * 128,
                                         lcol:lcol + nn])
                ps = psD.tile([128, NCOL], F32, tag="bmm", name="bmm")
                for k in range(kch):
                    nc.tensor.matmul(ps[:nn, :], lh[:, k, :nn],
                                     wg_sb[li][:, k, :],
                                     start=(k == 0), stop=(k == kch - 1))
                tb = bpool.tile([128, L["row"]], BF16, tag="btb", name="btb")
                nc.scalar.activation(tb[:nn, :NCOL], ps[:nn, :], AF.Copy)
                g0 = r * n_loc + tt * 128
                nc.sync.dma_start(
                    tblF[li][g0:g0 + nn, :NCOL], tb[:nn, :NCOL])

            def build_sched(li):
                # yields per-slot lists of (r, tt) build jobs, chunk-gated:
                # jobs of chunk k only in slots > bounds[k+1]-1; leftovers
                # returned for emission after the loop.
                pend = []
                nxt = {k: [(r, tt) for tt in range(bounds[k], bounds[k + 1])
                           for r in range(R)] for k in range(K)}
                sched = [[] for _ in range(W)]
                per = math.ceil(R * W / max(W - bounds[1], 1))
                avail = []
                for j in range(W):
                    for k in range(K):
                        if j == bounds[k + 1] - 1:
                            pass
                    # jobs of chunk k become available at slot bounds[k+1]
                    for k in range(K):
                        if j == bounds[k + 1]:
                            avail.extend(nxt[k])
                            nxt[k] = []
                    take = min(per, len(avail))
                    sched[j] = avail[:take]
                    avail = avail[take:]
                for k in range(K):
                    avail.extend(nxt[k])
                return sched, avail

            ad_tiles = {}

            def emit_ad(li):
                L = Ls[li]
                H = L["H"]
                ad_all = spool.tile([128, W, MAXH], BF16, tag="ad_all",
                                    name="ad_all", bufs=2)
                ad_tiles[li] = ad_all
                nc.vector.memset(ad_all[:], 0.0)
                full_w = n_loc // 128
                ad_f = tblS[li][:full_w * 128, L["ad_off"]:L["ad_off"] + H]
                nc.sync.dma_start(
                    ad_all[:, :full_w, :H],
                    ad_f.rearrange("(w q) h -> q w h", q=128))
                if n_loc % 128:
                    rem = n_loc - full_w * 128
                    nc.sync.dma_start(
                        ad_all[:rem, full_w, :H],
                        tblS[li][full_w * 128:, L["ad_off"]:L["ad_off"] + H])

            # ================= MLP + layer-0 tables =================
            sched0, left0 = build_sched(0)
            for j in range(W):
                n0 = j * 128
                nn = cfg.win_size(j)
                kc = cfg.chunk_of_win(j)
                lcol = n0 - starts[kc]
                xt = npool.tile([128, 2, 128], BF16, tag="xt")
                for k in range(2):
                    nc.sync.dma_start(xt[:, k, :nn],
                                      xT[k * 128:(k + 1) * 128, n0:n0 + nn])
                ps = psC.tile([128, 128], F32, tag="mm")
                for k in range(2):
                    nc.tensor.matmul(ps[:, :nn], wm1[:, k, :], xt[:, k, :nn],
                                     start=(k == 0), stop=(k == 1))
                h1 = npool.tile([128, 128], BF16, tag="h1")
                nc.scalar.activation(h1[:, :nn], ps[:, :nn], AF.Relu,
                                     bias=b1s[:, 0:1])
                ps2 = psC.tile([128, 128], F32, tag="mm")
                nc.tensor.matmul(ps2[:, :nn], wm2[:, :], h1[:, :nn],
                                 start=True, stop=True)
                h2 = npool.tile([128, 128], BF16, tag="h2")
                nc.scalar.activation(h2[:, :nn], ps2[:, :nn], AF.Relu,
                                     bias=b2s[:, 0:1])
                nc.sync.dma_start(piece_in[0][kc][:, lcol:lcol + nn],
                                  h2[:, :nn])
                node_self(0, j)
                for k in range(K):
                    if j == bounds[k + 1] - 1:
                        emit_ag(0, k)
                for (r, tt) in sched0[j]:
                    build_full(0, r, tt)
            for (r, tt) in left0:
                build_full(0, r, tt)
            emit_ad(0)

            # ================= layers =================
            for li in range(3):
                L = Ls[li]
                H, Cc, HC, ROW = L["H"], L["C"], L["HC"], L["row"]
                NCOL = HC + 2 * H
                ad_all = ad_tiles[li]
                if li < 2:
                    schedN, leftN = build_sched(li + 1)

                for t in range(W):
                    Tlo, Thi = p.T_lo[t], p.T_hi[t]
                    T = Tlo + Thi + 1
                    nn = cfg.win_size(t)
                    to = p.win_tile_off[t]
                    g = gpool.tile([128, T, ROW], BF16, tag="g")
                    # self-loop rows: direct sequential load from mini table
                    nc.sync.dma_start(
                        g[:nn, T - 1, :NCOL],
                        tblS[li][t * 128:t * 128 + nn, :NCOL])
                    GMAX = 8  # tiles per dma_gather (>1024 idxs crashes HW)
                    for q0 in range(0, Tlo, GMAX):
                        q = min(GMAX, Tlo - q0)
                        nc.gpsimd.dma_gather(
                            g[:, q0:q0 + q, :], tblF[li][:cfg.NSPLIT, :],
                            idx_lo[:, p.win_lo_off[t] + q0 * 8:
                                   p.win_lo_off[t] + (q0 + q) * 8],
                            q * 128, q * 128, ROW,
                            queue_num=gq[0] % cfg.n_queues)
                        gq[0] += 1
                    for q0 in range(0, Thi, GMAX):
                        q = min(GMAX, Thi - q0)
                        nc.gpsimd.dma_gather(
                            g[:, Tlo + q0:Tlo + q0 + q, :],
                            tblF[li][cfg.NSPLIT:, :],
                            idx_hi[:, p.win_hi_off[t] + q0 * 8:
                                   p.win_hi_off[t] + (q0 + q) * 8],
                            q * 128, q * 128, ROW,
                            queue_num=gq[0] % cfg.n_queues)
                        gq[0] += 1

                    oT = opool.tile([128, T, 128], F8, tag="oT")
                    oD = opool.tile([128, T, 128], F8, tag="oD")
                    nc.sync.dma_start(
                        oT.rearrange("p a b -> p (a b)"),
                        oT_d[:, to * 128:(to + T) * 128])
                    nc.sync.dma_start(
                        oD.rearrange("p a b -> p (a b)"),
                        oD_d[:, to * 128:(to + T) * 128])

                    ps_ad = psB.tile([128, MAXT * MAXH], F32, tag="ps_ad")
                    for i in range(T):
                        nc.tensor.matmul(ps_ad[:, i * H:(i + 1) * H],
                                         oD[:, i, :], ad_all[:, t, :H],
                                         start=True, stop=True)
                    e_sb = spool.tile([128, MAXT * MAXH], F32, tag="e_sb")
                    nc.vector.tensor_tensor(
                        e_sb[:, :T * H], ps_ad[:, :T * H],
                        g[:, 0:T, L["as_off"]:L["as_off"] + H],
                        ALU.add)
                    ex1 = spool.tile([128, MAXT * MAXH], F32, tag="ex1")
                    nc.scalar.activation(ex1[:, :T * H], e_sb[:, :T * H], AF.Exp)
                    ex2 = spool.tile([128, MAXT * MAXH], F32, tag="ex2")
                    nc.scalar.activation(ex2[:, :T * H], e_sb[:, :T * H], AF.Exp,
                                         scale=NEG_SLOPE)
                    msg = mpool.tile([128, T, HC + H], BF16, tag="msg")
                    nc.vector.tensor_tensor(
                        msg[:, 0:T, HC:HC + H],
                        ex1[:, :T * H], ex2[:, :T * H], ALU.max)
                    for k0 in range(0, T, cfg.K_FUSE):
                        Kf = min(cfg.K_FUSE, T - k0)
                        nc.vector.tensor_tensor(
                            msg[:, k0:k0 + Kf, 0:HC],
                            g[:, k0:k0 + Kf, 0:HC],
                            msg[:, k0:k0 + Kf, HC:HC + H]
                                .unsqueeze(3).broadcast_to([128, Kf, H, Cc]),
                            ALU.mult)
                    ps_w = psA.tile([128, HC + H], F32, tag="ps_w")
                    for i in range(T):
                        nc.tensor.matmul(ps_w[:, :], oT[:, i, :],
                                         msg[:, i, :],
                                         start=(i == 0), stop=(i == T - 1))
                    rcp = spool.tile([128, MAXH], F32, tag="rcp")
                    nc.vector.reciprocal(rcp[:, :H], ps_w[:, HC:HC + H])
                    if li == 1:
                        nc.scalar.activation(rcp[:, :H], rcp[:, :H], AF.Copy,
                                             scale=0.5)
                    y = spool.tile([128, MAXHC], F32, tag="y")
                    nc.vector.tensor_tensor(
                        y[:, :HC], ps_w[:, :HC],
                        rcp[:, :H].unsqueeze(2).broadcast_to([128, H, Cc]),
                        ALU.mult)
                    if li == 1:
                        nc.vector.tensor_tensor(y[:, :Cc], y[:, :Cc],
                                                y[:, Cc:2 * Cc], ALU.add)
                        ycols = Cc
                    else:
                        ycols = HC
                    nc.vector.tensor_tensor(
                        y[:, :ycols], y[:, :ycols],
                        bg_sb[li][:, :ycols], ALU.add)
                    if li < 2:
                        e1 = spool.tile([128, MAXHC], F32, tag="elu1")
                        nc.scalar.activation(e1[:, :ycols], y[:, :ycols], AF.Exp)
                        nc.scalar.activation(e1[:, :ycols], e1[:, :ycols],
                                             AF.Relu, scale=-1.0, bias=1.0)
                        nc.scalar.activation(y[:, :ycols], y[:, :ycols], AF.Relu)
                        yb = spool.tile([128, MAXHC], BF16, tag="yb")
                        nc.vector.tensor_tensor(yb[:, :ycols], y[:, :ycols],
                                                e1[:, :ycols], ALU.subtract)
                        kc = cfg.chunk_of_win(t)
                        lcol = t * 128 - starts[kc]
                        for k in range(ycols // 128):
                            pt = psC.tile([128, 128], BF16, tag="mm")
                            nc.tensor.transpose(pt[:, :],
                                                yb[:, k * 128:(k + 1) * 128],
                                                ident[:, :])
                            pts = spool.tile([128, 128], BF16, tag="pts")
                            nc.scalar.activation(pts[:, :], pt[:, :], AF.Copy)
                            nc.sync.dma_start(
                                piece_in[li + 1][kc][k * 128:(k + 1) * 128,
                                                     lcol:lcol + nn],
                                pts[:, :nn])
                    else:
                        nc.sync.dma_start(out_d[t * 128:t * 128 + nn, :],
                                          y[:nn, :ycols])

                    if li < 2:
                        node_self(li + 1, t)
                        for k in range(K):
                            if t == bounds[k + 1] - 1:
                                emit_ag(li + 1, k)
                        for (r, tt) in schedN[t]:
                            build_full(li + 1, r, tt)
                        if t == W - 1:
                            for (r, tt) in leftN:
                                build_full(li + 1, r, tt)
                            emit_ad(li + 1)
    return nc, out_d


# ---------------------------------------------------------------- runner

def make_inmaps(inputs, cfg: Cfg, p: Plan):
    wts = prep_weights(inputs, cfg)
    x = np.asarray(inputs["x"], np.float32)
    xT = np.ascontiguousarray(x.T).astype(bf)
    n_loc = cfg.n_loc
    in_maps = []
    for c in range(cfg.n_cores):
        m = dict(wts)
        m["xT"] = np.ascontiguousarray(xT[:, c * n_loc:(c + 1) * n_loc])
        m["idx_lo"] = p.idx_lo[c]
        m["idx_hi"] = p.idx_hi[c]
        m["oT"] = p.oT[c]
        m["oD"] = p.oD[c]
        in_maps.append(m)
    return in_maps


def build_program(cfg: Cfg, p: Plan, debug=False):
    nc = bacc.Bacc("TRN2", target_bir_lowering=False, debug=debug,
                   num_devices=cfg.n_cores, num_swdge_queues=cfg.n_queues)
    build(nc, cfg, p)
    nc.compile()
    return nc


# ------------------------------------------------------------- entry point

_CACHE = {}


def kernel(**inputs):
    import numpy as _np
    from concourse.bass_utils import run_bass_kernel_spmd

    cfg = Cfg()
    ei = _np.asarray(inputs["edge_index"])
    key = hash(ei.tobytes())
    if key not in _CACHE:
        p = plan_edges(ei, cfg)
        nc = build_program(cfg, p, debug=False)
        _CACHE[key] = (p, nc)
    p, nc = _CACHE[key]
    in_maps = make_inmaps(inputs, cfg, p)
    res = run_bass_kernel_spmd(nc, in_maps, list(range(cfg.n_cores)))
    out = _np.concatenate([res.results[c]["out"] for c in range(cfg.n_cores)], 0)
    return out.astype(_np.float32)
